# revision 4
# baseline (speedup 1.0000x reference)
"""Trainium2 Bass kernel for nn_CrossAttention (B=2, Nq=Nk=2048, H=8, Dh=64,
Dx=512, Dctx=768).

Sharding: (batch, q-block) across 8 cores — core c handles batch c//4, query
rows [(c%4)*512, (c%4+1)*512). Each core computes K/V projections for its
batch's full context (duplicated across the 4 cores sharing a batch), its own
Q slice, full softmax attention over all 2048 keys, and the output projection.
Output is fully local per core (no cross-core reduction).

All activations live feature-on-partition ("transposed") so every matmul
contracts along the SBUF partition axis. Host pre-transposes x/context and
pre-rounds all matmul inputs to float32r (11-bit mantissa) so DMA loads land
directly in f32r tiles.
"""

import sys

sys.path.insert(0, "/opt/trn_rl_repo")

import numpy as np

import concourse.bacc as bacc
import concourse.mybir as mybir
import concourse.tile as tile
from concourse.bass_utils import run_bass_kernel_spmd
from contextlib import ExitStack

F32 = mybir.dt.float32
F32R = mybir.dt.float32r

B = 2
NQ_FULL = 2048
NKV = 2048
DX = 512
DC = 768
DI = 512
NH = 8
DH = 64
NQ = 512  # q rows per core
N_CORES = 8

_CACHE = {}


def _build_nc():
    nc = bacc.Bacc("TRN2", target_bir_lowering=False, debug=False, num_devices=N_CORES)

    xt = nc.declare_dram_parameter("xt", [DX, NQ], F32R, isOutput=False)
    ctxt = nc.declare_dram_parameter("ctxt", [DC, NKV], F32R, isOutput=False)
    wq = nc.declare_dram_parameter("wq", [DX, DI], F32R, isOutput=False)
    wk = nc.declare_dram_parameter("wk", [DC, DI], F32R, isOutput=False)
    wv = nc.declare_dram_parameter("wv", [DC, DI], F32R, isOutput=False)
    wo = nc.declare_dram_parameter("wo", [DI, DI], F32R, isOutput=False)
    bo = nc.declare_dram_parameter("bo", [DI, 1], F32, isOutput=False)
    ot = nc.declare_dram_parameter("ot", [DI, NQ], F32, isOutput=True)

    KC_X = DX // 128  # 4 contraction chunks for x
    KC_C = DC // 128  # 6 contraction chunks for context
    MO = DI // 128  # 4 output-row chunks
    NKC = NKV // 128  # 16 kv chunks
    SCALE = DH ** -0.5

    with tile.TileContext(nc) as tc:
        with ExitStack() as ctx:
            # ---- SBUF pools ----
            const_p = ctx.enter_context(tc.tile_pool(name="const", bufs=1))
            w_p = ctx.enter_context(tc.tile_pool(name="weights", bufs=1))
            ctx_p = ctx.enter_context(tc.tile_pool(name="ctxt", bufs=1))
            kt_p = ctx.enter_context(tc.tile_pool(name="kt", bufs=1))
            vaug_p = ctx.enter_context(tc.tile_pool(name="vaug", bufs=1))
            qt_p = ctx.enter_context(tc.tile_pool(name="qt", bufs=1))
            at_p = ctx.enter_context(tc.tile_pool(name="at", bufs=1))
            pt_p = ctx.enter_context(tc.tile_pool(name="pt", bufs=4))
            small_p = ctx.enter_context(tc.tile_pool(name="small", bufs=1))
            out_p = ctx.enter_context(tc.tile_pool(name="outsb", bufs=2))
            # ---- PSUM pools ----
            acc_ps = ctx.enter_context(tc.tile_pool(name="acc_ps", bufs=4, space="PSUM"))
            attn_ps = ctx.enter_context(tc.tile_pool(name="attn_ps", bufs=2, space="PSUM"))
            bc_ps = ctx.enter_context(tc.tile_pool(name="bc_ps", bufs=2, space="PSUM"))

            # ---- constants ----
            ones_f = const_p.tile([128, 64], F32)
            nc.any.memset(ones_f[:], 1.0)
            ones_r = const_p.tile([128, 64], F32R)
            nc.vector.tensor_copy(ones_r[:], ones_f[:])
            ones32 = const_p.tile([128, 32], F32)
            nc.any.memset(ones32[:], 1.0)

            # ---- DMA inputs ----
            wq_t = []
            for c in range(KC_X):
                t = w_p.tile([128, DI], F32R, tag=f"wq{c}")
                nc.sync.dma_start(t[:], wq[c * 128:(c + 1) * 128, :])
                wq_t.append(t)
            wk_t = []
            wv_t = []
            for c in range(KC_C):
                t = w_p.tile([128, DI], F32R, tag=f"wk{c}")
                nc.sync.dma_start(t[:], wk[c * 128:(c + 1) * 128, :])
                wk_t.append(t)
                t = w_p.tile([128, DI], F32R, tag=f"wv{c}")
                nc.sync.dma_start(t[:], wv[c * 128:(c + 1) * 128, :])
                wv_t.append(t)
            wo_t = []
            for h in range(NH):
                t = w_p.tile([64, DI], F32R, tag=f"wo{h}")
                nc.sync.dma_start(t[:], wo[h * 64:(h + 1) * 64, :])
                wo_t.append(t)
            bo_t = []
            for m in range(MO):
                t = w_p.tile([128, 1], F32, tag=f"bo{m}")
                nc.sync.dma_start(t[:], bo[m * 128:(m + 1) * 128, :])
                bo_t.append(t)
            xt_t = []
            for c in range(KC_X):
                t = pt_p.tile([128, NQ], F32R, tag="pt", name=f"xt{c}")
                nc.sync.dma_start(t[:], xt[c * 128:(c + 1) * 128, :])
                xt_t.append(t)
            ctx_t = []
            for c in range(KC_C):
                t = ctx_p.tile([128, NKV], F32R, tag=f"ctx{c}")
                nc.sync.dma_start(t[:], ctxt[c * 128:(c + 1) * 128, :])
                ctx_t.append(t)

            # ---- Phase B: QT [di, q] = Wq^T @ x^T ----
            qt_t = []
            for m in range(MO):
                ps = acc_ps.tile([128, NQ], F32, tag="acc")
                for c in range(KC_X):
                    nc.tensor.matmul(
                        ps[:], wq_t[c][:, m * 128:(m + 1) * 128], xt_t[c][:],
                        start=(c == 0), stop=(c == KC_X - 1))
                t = qt_p.tile([128, NQ], F32R, tag=f"qt{m}")
                nc.scalar.copy(t[:], ps[:])
                qt_t.append(t)

            # ---- Phase C: KT [di, kv] = Wk^T @ ctx^T ----
            kt_t = [kt_p.tile([128, NKV], F32R, tag=f"kt{m}", name=f"kt{m}") for m in range(MO)]
            for m in range(MO):
                pss = [acc_ps.tile([128, 512], F32, tag="acc", name=f"pkt{m}_{n}") for n in range(4)]
                for c in range(KC_C):
                    for n in range(4):
                        nc.tensor.matmul(
                            pss[n][:], wk_t[c][:, m * 128:(m + 1) * 128],
                            ctx_t[c][:, n * 512:(n + 1) * 512],
                            start=(c == 0), stop=(c == KC_C - 1))
                for n in range(4):
                    nc.scalar.copy(kt_t[m][:, n * 512:(n + 1) * 512], pss[n][:])

            # ---- Phase D: V_aug [kv, 8*(64+1)] = ctx @ Wv (+ ones col per head) ----
            vaug_t = [vaug_p.tile([128, 4 * 520], F32R, tag=f"va{i}", name=f"va{i}") for i in range(4)]
            for i in range(4):
                dst_ones = vaug_t[i][:].rearrange("p (g c) -> p g c", c=65)[:, :, 64:65]
                nc.vector.tensor_copy(dst_ones, ones32[:, :, None])
            for kvc in range(NKC):
                ps = acc_ps.tile([128, DI], F32, tag="acc")
                for c in range(KC_C):
                    nc.tensor.matmul(
                        ps[:], ctx_t[c][:, kvc * 128:(kvc + 1) * 128], wv_t[c][:],
                        start=(c == 0), stop=(c == KC_C - 1))
                dst = vaug_t[kvc // 4][:, (kvc % 4) * 520:(kvc % 4 + 1) * 520]
                dst = dst.rearrange("p (h c) -> p h c", c=65)[:, :, 0:64]
                src = ps[:].rearrange("p (h c) -> p h c", c=64)
                nc.vector.tensor_copy(dst, src)

            # ---- Phase E: attention per head ----
            at_t = []
            for h in range(NH):
                hm, ho = h // 2, (h % 2) * 64
                ps_a = attn_ps.tile([65, NQ], F32, tag="attn")
                for kvc in range(NKC):
                    ps_s = acc_ps.tile([128, NQ], F32, tag="acc")
                    nc.tensor.matmul(
                        ps_s[:],
                        kt_t[hm][ho:ho + 64, kvc * 128:(kvc + 1) * 128],
                        qt_t[hm][ho:ho + 64, :],
                        start=True, stop=True)
                    p_t = pt_p.tile([128, NQ], F32R, tag="pt")
                    nc.scalar.activation(p_t[:], ps_s[:],
                                         mybir.ActivationFunctionType.Exp,
                                         scale=SCALE)
                    va = vaug_t[kvc // 4][:, (kvc % 4) * 520 + h * 65:
                                          (kvc % 4) * 520 + (h + 1) * 65]
                    nc.tensor.matmul(ps_a[:], va, p_t[:],
                                     start=(kvc == 0), stop=(kvc == NKC - 1))
                # normalize: recip of denom row (partition 64, stays aligned)
                rec = small_p.tile([65, NQ], F32, tag="rec")
                nc.vector.reciprocal(rec[64:65, :], ps_a[64:65, :])
                rec_r = small_p.tile([65, NQ], F32R, tag="recr")
                nc.vector.tensor_copy(rec_r[64:65, :], rec[64:65, :])
                ps_b = bc_ps.tile([64, NQ], F32, tag="bc")
                nc.tensor.matmul(ps_b[:], ones_r[64:65, 0:64], rec_r[64:65, :],
                                 start=True, stop=True)
                b_sb = small_p.tile([64, NQ], F32, tag="bsb")
                nc.scalar.copy(b_sb[:], ps_b[:])
                a_t = at_p.tile([64, NQ], F32R, tag=f"at{h}")
                nc.vector.tensor_tensor(a_t[:], ps_a[0:64, :], b_sb[:],
                                        op=mybir.AluOpType.mult)
                at_t.append(a_t)

            # ---- Phase F: OT [di, q] = Wo^T @ attn^T + bo ----
            for m in range(MO):
                ps = acc_ps.tile([128, NQ], F32, tag="acc")
                for h in range(NH):
                    nc.tensor.matmul(ps[:], wo_t[h][:, m * 128:(m + 1) * 128],
                                     at_t[h][:],
                                     start=(h == 0), stop=(h == NH - 1))
                o_sb = out_p.tile([128, NQ], F32, tag="osb")
                nc.vector.tensor_scalar_add(o_sb[:], ps[:], bo_t[m][:])
                nc.sync.dma_start(ot[m * 128:(m + 1) * 128, :], o_sb[:])

    nc.finalize()
    return nc


def _round_f32r(a):
    """Round fp32 to f32r (11 explicit mantissa bits), round-to-nearest."""
    bits = np.ascontiguousarray(a, dtype=np.float32).view(np.uint32).astype(np.uint64)
    bits = (bits + 0x800) & 0xFFFFF000
    return bits.astype(np.uint32).view(np.float32)


def run_spmd(inputs, trace=False):
    if "nc" not in _CACHE:
        _CACHE["nc"] = _build_nc()
    nc = _CACHE["nc"]

    x = np.asarray(inputs["x"], dtype=np.float32)
    context = np.asarray(inputs["context"], dtype=np.float32)
    wq_r = _round_f32r(inputs["Wq"])
    wk_r = _round_f32r(inputs["Wk"])
    wv_r = _round_f32r(inputs["Wv"])
    wo_r = _round_f32r(inputs["Wo"])
    bo2 = np.ascontiguousarray(np.asarray(inputs["bo"], np.float32).reshape(DI, 1))

    ctxt_b = [_round_f32r(np.ascontiguousarray(context[b].T)) for b in range(B)]
    in_maps = []
    for c in range(N_CORES):
        b, q0 = c // 4, (c % 4) * NQ
        xt_c = _round_f32r(np.ascontiguousarray(x[b, q0:q0 + NQ, :].T))
        in_maps.append({
            "xt": xt_c, "ctxt": ctxt_b[b],
            "wq": wq_r, "wk": wk_r, "wv": wv_r, "wo": wo_r, "bo": bo2,
        })

    res = run_bass_kernel_spmd(nc, in_maps, core_ids=list(range(N_CORES)),
                               trace=trace)
    out = np.empty((B, NQ_FULL, DI), dtype=np.float32)
    for c in range(N_CORES):
        b, q0 = c // 4, (c % 4) * NQ
        out[b, q0:q0 + NQ, :] = res.results[c]["ot"].T
    return out, res


def kernel(**inputs):
    out, _ = run_spmd(inputs, trace=False)
    return out


# revision 6
# speedup vs baseline: 1.1896x; 1.1896x over previous
"""Trainium2 Bass kernel for nn_CrossAttention (B=2, Nq=Nk=2048, H=8, Dh=64,
Dx=512, Dctx=768).

Sharding: (batch, q-block) across 8 cores — core c handles batch c//4, query
rows [(c%4)*512, (c%4+1)*512). Each core computes K/V projections for its
batch's full context (duplicated across the 4 cores sharing a batch), its own
Q slice, full softmax attention over all 2048 keys, and the output projection.
Output is fully local per core (no cross-core reduction).

All activations live feature-on-partition ("transposed") so every matmul
contracts along the SBUF partition axis. Matmul operands are bf16 (PSUM
accumulation is fp32); host pre-casts/transposes the inputs. Softmax runs on
S^T without max-subtraction (scores are ~N(0,1)); denominators come from a
ones-column appended to V, so attention + normalization constants fall out of
one PV accumulation chain.
"""

import os
import sys

sys.path.insert(0, "/opt/trn_rl_repo")

import numpy as np
import ml_dtypes

import concourse.bacc as bacc
import concourse.mybir as mybir
import concourse.tile as tile
from concourse.bass_utils import run_bass_kernel_spmd
from contextlib import ExitStack

F32 = mybir.dt.float32
BF16 = mybir.dt.bfloat16
NP_BF16 = np.dtype(ml_dtypes.bfloat16)

B = 2
NQ_FULL = 2048
NKV = 2048
DX = 512
DC = 768
DI = 512
NH = 8
DH = 64
NQ = 512  # q rows per core
N_CORES = 8

_CACHE = {}


def _build_nc():
    nc = bacc.Bacc("TRN2", target_bir_lowering=False, debug=False, num_devices=N_CORES)

    xt = nc.declare_dram_parameter("xt", [DX, NQ], BF16, isOutput=False)
    ctxt = nc.declare_dram_parameter("ctxt", [DC, NKV], BF16, isOutput=False)
    wq = nc.declare_dram_parameter("wq", [DX, DI], BF16, isOutput=False)
    wk = nc.declare_dram_parameter("wk", [DC, DI], BF16, isOutput=False)
    wv = nc.declare_dram_parameter("wv", [DC, DI], BF16, isOutput=False)
    wo = nc.declare_dram_parameter("wo", [DI, DI], BF16, isOutput=False)
    bo = nc.declare_dram_parameter("bo", [DI, 1], F32, isOutput=False)
    ot = nc.declare_dram_parameter("ot", [DI, NQ], F32, isOutput=True)

    KC_X = DX // 128  # 4 contraction chunks for x
    KC_C = DC // 128  # 6 contraction chunks for context
    MO = DI // 128  # 4 output-row chunks
    NKC = NKV // 128  # 16 kv chunks
    SCALE = DH ** -0.5

    with tile.TileContext(nc) as tc:
        with ExitStack() as ctx:
            # ---- SBUF pools ----
            const_p = ctx.enter_context(tc.tile_pool(name="const", bufs=1))
            w_p = ctx.enter_context(tc.tile_pool(name="weights", bufs=1))
            ctx_p = ctx.enter_context(tc.tile_pool(name="ctxt", bufs=1))
            kt_p = ctx.enter_context(tc.tile_pool(name="kt", bufs=1))
            vaug_p = ctx.enter_context(tc.tile_pool(name="vaug", bufs=1))
            qt_p = ctx.enter_context(tc.tile_pool(name="qt", bufs=1))
            at_p = ctx.enter_context(tc.tile_pool(name="at", bufs=1))
            pt_p = ctx.enter_context(tc.tile_pool(name="pt", bufs=6))
            small_p = ctx.enter_context(tc.tile_pool(name="small", bufs=2))
            out_p = ctx.enter_context(tc.tile_pool(name="outsb", bufs=2))
            # ---- PSUM pools ----
            acc_ps = ctx.enter_context(tc.tile_pool(name="acc_ps", bufs=4, space="PSUM"))
            attn_ps = ctx.enter_context(tc.tile_pool(name="attn_ps", bufs=2, space="PSUM"))
            bc_ps = ctx.enter_context(tc.tile_pool(name="bc_ps", bufs=2, space="PSUM"))

            # ---- constants ----
            ones_f = const_p.tile([128, 64], F32)
            nc.any.memset(ones_f[:], 1.0)
            ones_r = const_p.tile([128, 64], BF16)
            nc.vector.tensor_copy(ones_r[:], ones_f[:])
            ones32 = const_p.tile([128, 32], F32)
            nc.any.memset(ones32[:], 1.0)

            # ---- DMA inputs ----
            wq_t = []
            for c in range(KC_X):
                t = w_p.tile([128, DI], BF16, tag=f"wq{c}")
                nc.sync.dma_start(t[:], wq[c * 128:(c + 1) * 128, :])
                wq_t.append(t)
            wk_t = []
            wv_t = []
            for c in range(KC_C):
                t = w_p.tile([128, DI], BF16, tag=f"wk{c}")
                nc.sync.dma_start(t[:], wk[c * 128:(c + 1) * 128, :])
                wk_t.append(t)
                t = w_p.tile([128, DI], BF16, tag=f"wv{c}")
                nc.sync.dma_start(t[:], wv[c * 128:(c + 1) * 128, :])
                wv_t.append(t)
            wo_t = []
            for h in range(NH):
                t = w_p.tile([64, DI], BF16, tag=f"wo{h}")
                nc.sync.dma_start(t[:], wo[h * 64:(h + 1) * 64, :])
                wo_t.append(t)
            bo_t = []
            for m in range(MO):
                t = w_p.tile([128, 1], F32, tag=f"bo{m}")
                nc.sync.dma_start(t[:], bo[m * 128:(m + 1) * 128, :])
                bo_t.append(t)
            xt_t = []
            for c in range(KC_X):
                t = pt_p.tile([128, NQ], BF16, tag="pt", name=f"xt{c}")
                nc.sync.dma_start(t[:], xt[c * 128:(c + 1) * 128, :])
                xt_t.append(t)
            ctx_t = []
            for c in range(KC_C):
                t = ctx_p.tile([128, NKV], BF16, tag=f"ctx{c}")
                nc.sync.dma_start(t[:], ctxt[c * 128:(c + 1) * 128, :])
                ctx_t.append(t)

            # ---- Phase B: QT [di, q] = Wq^T @ x^T ----
            qt_t = []
            for m in range(MO):
                ps = acc_ps.tile([128, NQ], F32, tag="acc")
                for c in range(KC_X):
                    nc.tensor.matmul(
                        ps[:], wq_t[c][:, m * 128:(m + 1) * 128], xt_t[c][:],
                        start=(c == 0), stop=(c == KC_X - 1))
                t = qt_p.tile([128, NQ], BF16, tag=f"qt{m}")
                nc.scalar.copy(t[:], ps[:])
                qt_t.append(t)

            # ---- Phase C: KT [di, kv] = Wk^T @ ctx^T (N=1024 moving slices) ----
            kt_t = [kt_p.tile([128, NKV], BF16, tag=f"kt{m}", name=f"kt{m}") for m in range(MO)]
            for m in range(MO):
                pss = [acc_ps.tile([128, 512], F32, tag="acc", name=f"pkt{m}_{n}") for n in range(4)]
                for c in range(KC_C):
                    for n in range(4):
                        nc.tensor.matmul(
                            pss[n][:], wk_t[c][:, m * 128:(m + 1) * 128],
                            ctx_t[c][:, n * 512:(n + 1) * 512],
                            start=(c == 0), stop=(c == KC_C - 1))
                for n in range(4):
                    nc.scalar.copy(kt_t[m][:, n * 512:(n + 1) * 512], pss[n][:])

            # ---- Phase D: V_aug [kv, 8*(64+1)] = ctx @ Wv (+ ones col per head) ----
            vaug_t = [vaug_p.tile([128, 4 * 520], BF16, tag=f"va{i}", name=f"va{i}") for i in range(4)]
            for i in range(4):
                dst_ones = vaug_t[i][:].rearrange("p (g c) -> p g c", c=65)[:, :, 64:65]
                nc.vector.tensor_copy(dst_ones, ones32[:, :, None])
            for kvc in range(NKC):
                ps = acc_ps.tile([128, DI], F32, tag="acc")
                for c in range(KC_C):
                    nc.tensor.matmul(
                        ps[:], ctx_t[c][:, kvc * 128:(kvc + 1) * 128], wv_t[c][:],
                        start=(c == 0), stop=(c == KC_C - 1))
                dst = vaug_t[kvc // 4][:, (kvc % 4) * 520:(kvc % 4 + 1) * 520]
                dst = dst.rearrange("p (h c) -> p h c", c=65)[:, :, 0:64]
                src = ps[:].rearrange("p (h c) -> p h c", c=64)
                nc.vector.tensor_copy(dst, src)

            # ---- Phase E: attention per head ----
            at_t = []
            for h in range(NH):
                hm, ho = h // 2, (h % 2) * 64
                ps_a = attn_ps.tile([65, NQ], F32, tag="attn")
                for kvc in range(NKC):
                    ps_s = acc_ps.tile([128, NQ], F32, tag="acc")
                    nc.tensor.matmul(
                        ps_s[:],
                        kt_t[hm][ho:ho + 64, kvc * 128:(kvc + 1) * 128],
                        qt_t[hm][ho:ho + 64, :],
                        start=True, stop=True)
                    p_t = pt_p.tile([128, NQ], BF16, tag="pt")
                    nc.scalar.activation(p_t[:], ps_s[:],
                                         mybir.ActivationFunctionType.Exp,
                                         scale=SCALE)
                    va = vaug_t[kvc // 4][:, (kvc % 4) * 520 + h * 65:
                                          (kvc % 4) * 520 + (h + 1) * 65]
                    nc.tensor.matmul(ps_a[:], va, p_t[:],
                                     start=(kvc == 0), stop=(kvc == NKC - 1))
                # normalize: recip of denom row (partition 64, stays aligned)
                rec = small_p.tile([65, NQ], F32, tag="rec")
                nc.vector.reciprocal(rec[64:65, :], ps_a[64:65, :])
                rec_r = small_p.tile([65, NQ], BF16, tag="recr")
                nc.vector.tensor_copy(rec_r[64:65, :], rec[64:65, :])
                ps_b = bc_ps.tile([64, NQ], F32, tag="bc")
                nc.tensor.matmul(ps_b[:], ones_r[64:65, 0:64], rec_r[64:65, :],
                                 start=True, stop=True)
                b_sb = small_p.tile([64, NQ], F32, tag="bsb")
                nc.scalar.copy(b_sb[:], ps_b[:])
                a_t = at_p.tile([64, NQ], BF16, tag=f"at{h}")
                nc.vector.tensor_tensor(a_t[:], ps_a[0:64, :], b_sb[:],
                                        op=mybir.AluOpType.mult)
                at_t.append(a_t)

            # ---- Phase F: OT [di, q] = Wo^T @ attn^T + bo ----
            for m in range(MO):
                ps = acc_ps.tile([128, NQ], F32, tag="acc")
                for h in range(NH):
                    nc.tensor.matmul(ps[:], wo_t[h][:, m * 128:(m + 1) * 128],
                                     at_t[h][:],
                                     start=(h == 0), stop=(h == NH - 1))
                o_sb = out_p.tile([128, NQ], F32, tag="osb")
                nc.vector.tensor_scalar_add(o_sb[:], ps[:], bo_t[m][:])
                nc.sync.dma_start(ot[m * 128:(m + 1) * 128, :], o_sb[:])

    nc.finalize()
    return nc


def _bf16(a):
    return np.ascontiguousarray(a).astype(NP_BF16)


def run_spmd(inputs, trace=False):
    if "nc" not in _CACHE:
        _CACHE["nc"] = _build_nc()
    nc = _CACHE["nc"]

    x = np.asarray(inputs["x"], dtype=np.float32)
    context = np.asarray(inputs["context"], dtype=np.float32)
    wq_r = _bf16(inputs["Wq"])
    wk_r = _bf16(inputs["Wk"])
    wv_r = _bf16(inputs["Wv"])
    wo_r = _bf16(inputs["Wo"])
    bo2 = np.ascontiguousarray(np.asarray(inputs["bo"], np.float32).reshape(DI, 1))

    ctxt_b = [_bf16(context[b].T) for b in range(B)]
    in_maps = []
    for c in range(N_CORES):
        b, q0 = c // 4, (c % 4) * NQ
        xt_c = _bf16(x[b, q0:q0 + NQ, :].T)
        in_maps.append({
            "xt": xt_c, "ctxt": ctxt_b[b],
            "wq": wq_r, "wk": wk_r, "wv": wv_r, "wo": wo_r, "bo": bo2,
        })

    res = run_bass_kernel_spmd(nc, in_maps, core_ids=list(range(N_CORES)),
                               trace=trace)
    out = np.empty((B, NQ_FULL, DI), dtype=np.float32)
    for c in range(N_CORES):
        b, q0 = c // 4, (c % 4) * NQ
        out[b, q0:q0 + NQ, :] = res.results[c]["ot"].T
    return out, res


def kernel(**inputs):
    out, _ = run_spmd(inputs, trace=False)
    return out
